# revision 2
# baseline (speedup 1.0000x reference)
"""NatAttention (dilated/strided grouped neighborhood attention) on 8 NeuronCores.

Sharding: 96 H-rows split 8 ways (12 query rows / core) with a 15-row halo
(attention reach 14 + 1 conv row). Each core computes the depthwise qkv conv
as 9 shifted multiply-adds (channel source-gather done at shard-prep time),
then block-level (2x2 queries share a window) neighborhood attention with
static index tables, then the 1x1 projection. Only simple XLA ops are used
(slices, takes, batched matmuls) so it compiles cleanly for the neuron
backend.
"""
import numpy as np
import jax
import jax.numpy as jnp
from functools import partial

DIM = 128
HEADS = 8
HD = DIM // HEADS
KS = 8
STRIDE = 2
DIL = 2
H = W = 96
NC = 8
RP = H // NC                 # 12 query rows per core
HALO = (KS - 1) * DIL + 1    # 15
SR = RP + 2 * HALO           # 42 slice rows
MB = RP // 2                 # 6 block rows per core
NB = W // 2                  # 48 block cols


def _starts(L):
    # block index b -> window start (sub-grid units), clamped
    Ls = L // DIL
    return np.array([min(max(b - (KS - 1) // 2, 0), Ls - KS) for b in range(L // STRIDE)], np.int32)


@partial(jax.pmap, axis_name="x")
def _core(xq, xk, xv, wq, wk, wv, bq, bk, bv, ri, ci, pw, pb):
    # xq/xk/xv: [128, SR, W] pre-gathered conv inputs; wq..: [128, 3, 3]
    # ri: [MB*KS] local k-row indices; ci: [NB*KS] k-col indices
    def dwconv(xs, w, b):
        xp = jnp.pad(xs, ((0, 0), (1, 1), (1, 1)))
        acc = b[:, None, None] * jnp.ones((DIM, SR, W), jnp.float32)
        for dy in range(3):
            for dx in range(3):
                acc = acc + w[:, dy, dx, None, None] * \
                    jax.lax.slice(xp, (0, dy, dx), (DIM, dy + SR, dx + W))
        return acc

    q = dwconv(xq, wq, bq)[:, HALO:HALO + RP]        # [128, 12, W]
    k = dwconv(xk, wk, bk)                            # [128, SR, W]
    v = dwconv(xv, wv, bv)

    def heads(t):  # [128, R, W] -> [R, W, nh, hd]
        return t.reshape(HEADS, HD, t.shape[1], W).transpose(2, 3, 0, 1)

    q = heads(q)
    k = heads(k)
    v = heads(v)

    # block queries: [MB, 2, NB, 2, nh, hd] -> [MB, NB, nh, 4, hd]
    qb = q.reshape(MB, 2, NB, 2, HEADS, HD).transpose(0, 2, 4, 1, 3, 5)
    qb = qb.reshape(MB, NB, HEADS, 4, HD) * (HD ** -0.5)

    def windows(t):  # gather [MB, NB, nh, KS*KS, hd] K/V windows
        tr = jnp.take(t, ri, axis=0).reshape(MB, KS, W, HEADS, HD)
        trc = jnp.take(tr, ci, axis=2).reshape(MB, KS, NB, KS, HEADS, HD)
        return trc.transpose(0, 2, 4, 1, 3, 5).reshape(MB, NB, HEADS, KS * KS, HD)

    kw = windows(k)
    vw = windows(v)
    logits = jnp.einsum("bcnqd,bcnwd->bcnqw", qb, kw)        # [MB,NB,nh,4,64]
    attn = jax.nn.softmax(logits, axis=-1)
    ob = jnp.einsum("bcnqw,bcnwd->bcnqd", attn, vw)          # [MB,NB,nh,4,hd]
    # unblock -> [nh, hd, 12, W] -> [128, 12, W]
    oc = ob.reshape(MB, NB, HEADS, 2, 2, HD).transpose(2, 5, 0, 3, 1, 4)
    oc = oc.reshape(DIM, RP, W)
    out = jnp.einsum("oc,chw->ohw", pw, oc.astype(jnp.float32)) + pb[:, None, None]
    return out


def kernel(x, qkv_w, qkv_b, proj_w, proj_b):
    x = np.asarray(x, np.float32)
    qkv_w = np.asarray(qkv_w, np.float32)
    qkv_b = np.asarray(qkv_b, np.float32)
    src = np.arange(384) // 3                      # grouped-conv input channel
    w3 = qkv_w[:, 0]                               # [384, 3, 3]

    rs = _starts(H)                                # [48] row-block starts
    cs = _starts(W)
    ci = (DIL * (cs[:, None] + np.arange(KS)[None])).astype(np.int32).reshape(-1)

    XQ = np.zeros((NC, DIM, SR, W), np.float32)
    XK = np.zeros_like(XQ)
    XV = np.zeros_like(XQ)
    RI = np.zeros((NC, MB * KS), np.int32)
    for c in range(NC):
        r0 = c * RP
        lo, hi = r0 - HALO, r0 + RP + HALO
        slo, shi = max(lo, 0), min(hi, H)
        for arr, ch0 in ((XQ, 0), (XK, DIM), (XV, 2 * DIM)):
            arr[c, :, slo - lo:shi - lo] = x[0, src[ch0:ch0 + DIM], slo:shi]
        mb0 = r0 // 2
        RI[c] = (DIL * (rs[mb0:mb0 + MB, None] + np.arange(KS)[None]) - lo).astype(np.int32).reshape(-1)

    rep = lambda a: np.broadcast_to(np.ascontiguousarray(a, np.float32), (NC,) + np.asarray(a).shape)
    out = _core(XQ, XK, XV,
                rep(w3[:DIM]), rep(w3[DIM:2 * DIM]), rep(w3[2 * DIM:]),
                rep(qkv_b[:DIM]), rep(qkv_b[DIM:2 * DIM]), rep(qkv_b[2 * DIM:]),
                RI, np.broadcast_to(ci, (NC,) + ci.shape),
                rep(proj_w[:, :, 0, 0]), rep(proj_b))
    out = np.asarray(out)                          # [8, 128, 12, 96]
    return out.transpose(1, 0, 2, 3).reshape(1, DIM, H, W).astype(np.float32)
